# revision 1
# baseline (speedup 1.0000x reference)
"""Trainium2 Bass kernel for multi-head global attention (the
"DeformableAttention" module whose relative-position-bias path is inactive).

Reference computation (per batch b):
    qkv = x @ w_qkv.T + b_qkv            # [N, 3C]
    q, k, v = split/reshape to [nh, N, hd]
    attn = softmax((q @ k.T) * hd**-0.5)
    out  = (attn @ v) merged heads       # [N, C]
    y    = out @ w_proj.T + b_proj

Sharding: data-parallel over batch B=16 across 8 NeuronCores (2 batches/core).
No collectives.

Device-side design (per core, per batch):
  * x is staged pre-transposed (xT, [C, tokens]) so every matmul contraction
    dim lands on SBUF partitions without any on-device transpose.
  * Q^T, K^T ([hd, N]) are produced per-head straight from the QKV projection
    (head-sized M=96 stationary tiles); V in natural [N, nh*(hd+1)] layout
    with an interleaved ones-column per head (built by a rank-1 bias matmul).
  * Scores are computed transposed (S^T[k, q] blocks), softmax's exp runs on
    ScalarE with the 1/sqrt(hd) scale fused, and the row-sums fall out of the
    P~ @ [V | 1] matmul for free (row hd of the PSUM output).
  * O^T is copied out of PSUM immediately (frees the accumulator), normalized
    by the broadcast reciprocal row-sum (partition-broadcast via a DRAM
    bounce + stride-0 DMA), and DMA-repacked into a dense [C, N] attn^T
    buffer (partition-shifting SBUF->SBUF DMA).
  * Output projection contracts attn^T against w_proj.T in 6 dense 128-chunks,
    producing final [token, C] tiles in natural layout for direct DMA out.

All matmul operands are float32r (fp32 bit layout; the PE processes it at
bf16 rate for free dims >= 256, ~4x faster than strict fp32). Set
KERNEL_MM_DT=f32 for exact-fp32 matmuls instead.
"""

import os
import sys

sys.path.insert(0, "/opt/trn_rl_repo")

# The Bass->PJRT execution path needs jax to discover the axon-tunneled
# NeuronCores; a stray JAX_PLATFORMS=cpu (e.g. set for a jax reference run)
# would hide them. Only effective if jax hasn't been imported yet.
if "jax" not in sys.modules and "axon" not in os.environ.get("JAX_PLATFORMS", "axon"):
    os.environ.pop("JAX_PLATFORMS", None)

import numpy as np

import concourse.bass as bass
import concourse.mybir as mybir
import concourse.tile as tile
from concourse import bacc
from concourse.bass_utils import run_bass_kernel_spmd

# Problem constants (hardcoded per the task contract).
B, N, C = 16, 1024, 768
NH, HD = 8, 96
NCORES = 8
BPC = B // NCORES  # batches per core = 2
CC = C // 128  # contraction chunks of 128 = 6
KC = N // 128  # key chunks per batch = 8
QH = N // 512  # query halves = 2
TOKC = N // 128  # token chunks for V projection = 8
QC = N // 128  # query chunks for output projection = 8
HDA = HD + 1  # head dim + ones column = 97
VW = NH * HDA  # augmented V width = 776
SCALE = float(HD) ** -0.5

F32 = mybir.dt.float32

_BUILD_CACHE = {}


def _mm_dt():
    return F32 if os.environ.get("KERNEL_MM_DT") == "f32" else mybir.dt.float32r


def _build(qk_bias: bool, p_bias: bool):
    """Build + compile the single-core Bass program (shared SPMD across cores)."""
    knobs = tuple(
        int(os.environ.get(k, d))
        for k, d in (
            ("PT_BUFS", 3),
            ("QKT_BUFS", 2),
            ("WQKH_BUFS", 2),
            ("SP_BUFS", 2),
            ("OP_BUFS", 2),
            ("MP_BUFS", 2),
            ("RB_BUFS", 1),
            ("OUT_BUFS", 2),
            ("OT_BUFS", 2),
            ("ON_BUFS", 2),
        )
    )
    key = (qk_bias, p_bias, os.environ.get("KERNEL_MM_DT", "f32r"), knobs)
    if key in _BUILD_CACHE:
        return _BUILD_CACHE[key]
    ptb, qktb, wqkhb, spb, opb, mpb, rbb, outb, otb, onb = knobs

    mmdt = _mm_dt()

    nc = bacc.Bacc("TRN2", target_bir_lowering=False, debug=False)

    xT_d = nc.dram_tensor("xT", [C, BPC * N], mmdt, kind="ExternalInput")
    wqk_d = nc.dram_tensor("wqk", [C, 2 * NH * HD], mmdt, kind="ExternalInput")
    wv_d = nc.dram_tensor("wv", [C, VW], mmdt, kind="ExternalInput")
    wp_d = nc.dram_tensor("wp", [C, C], mmdt, kind="ExternalInput")
    bvaug_d = nc.dram_tensor("bvaug", [1, VW], mmdt, kind="ExternalInput")
    ones_d = nc.dram_tensor("ones", [1, 128], mmdt, kind="ExternalInput")
    vones_d = nc.dram_tensor("vones", [128, TOKC, NH], mmdt, kind="ExternalInput")
    if qk_bias:
        bqk_d = nc.dram_tensor("bqk", [HD, 2 * NH], F32, kind="ExternalInput")
    if p_bias:
        bp_d = nc.dram_tensor("bp", [1, C], mmdt, kind="ExternalInput")
    y_d = nc.dram_tensor("y", [BPC, N, C], F32, kind="ExternalOutput")

    xT_re = xT_d.rearrange("(o p) t -> p o t", p=128)
    wqk_re = wqk_d.rearrange("(o p) f -> p o f", p=128)
    wv_re = wv_d.rearrange("(o p) f -> p o f", p=128)
    wp_re = wp_d.rearrange("(o p) f -> p o f", p=128)

    EXP = mybir.ActivationFunctionType.Exp

    with tile.TileContext(nc) as tc:
        with (
            tc.tile_pool(name="wpool", bufs=1) as wpool,
            tc.tile_pool(name="wqkh_pool", bufs=wqkhb) as wqkh_pool,
            tc.tile_pool(name="xpool", bufs=1) as xpool,
            tc.tile_pool(name="qkt_pool", bufs=qktb) as qkt_pool,
            tc.tile_pool(name="vpool", bufs=1) as vpool,
            tc.tile_pool(name="pt_pool", bufs=ptb) as pt_pool,
            tc.tile_pool(name="attn_pool", bufs=2) as attn_pool,
            tc.tile_pool(name="rb_pool", bufs=rbb) as rb_pool,
            tc.tile_pool(name="ot_pool", bufs=otb) as ot_pool,
            tc.tile_pool(name="on_pool", bufs=onb) as on_pool,
            tc.tile_pool(name="rdram_pool", bufs=2, space="DRAM") as rdram_pool,
            tc.tile_pool(name="out_pool", bufs=outb) as out_pool,
            tc.tile_pool(name="spsum", bufs=spb, space="PSUM") as spsum,
            tc.tile_pool(name="opsum_pool", bufs=opb, space="PSUM") as opsum_pool,
            tc.tile_pool(name="mpsum", bufs=mpb, space="PSUM") as mpsum,
        ):
            # --- resident weights/constants ---
            wv_sb = wpool.tile([128, CC, VW], mmdt, tag="wv")
            nc.scalar.dma_start(wv_sb[:, 0:3, 0:512], wv_re[:, 0:3, 0:512])
            nc.scalar.dma_start(wv_sb[:, 3:CC, 0:512], wv_re[:, 3:CC, 0:512])
            nc.scalar.dma_start(wv_sb[:, :, 512:VW], wv_re[:, :, 512:VW])
            # wp is not needed until the first output projection (~100us in);
            # issue its load after the first batch's V projection to keep the
            # startup-critical DMAs (x, wv, wqk head 0) ahead of it.
            wp_sb = wpool.tile([128, CC, C], mmdt, tag="wp")
            bvaug_sb = wpool.tile([1, VW], mmdt, tag="bvaug")
            nc.scalar.dma_start(bvaug_sb[:], bvaug_d[:])
            ones_sb = wpool.tile([1, 128], mmdt, tag="ones")
            nc.scalar.dma_start(ones_sb[:], ones_d[:])
            if qk_bias:
                bqk_sb = wpool.tile([HD, 2 * NH], F32, tag="bqk")
                nc.scalar.dma_start(bqk_sb[:], bqk_d[:])
            if p_bias:
                bp_sb = wpool.tile([1, C], mmdt, tag="bp")
                nc.scalar.dma_start(bp_sb[:], bp_d[:])

            def emit_vproj(b):
                """Stage batch b's x^T and project V (ones-augmented)."""
                xTb = xpool.tile([128, CC, N], mmdt, tag="xTb", name="xTb")
                for xh in range(4):
                    nc.sync.dma_start(
                        xTb[:, :, xh * (N // 4) : (xh + 1) * (N // 4)],
                        xT_re[:, :, b * N + xh * (N // 4) : b * N + (xh + 1) * (N // 4)],
                    )
                v_sb = vpool.tile([128, TOKC, VW], mmdt, tag="v", name="v_sb")
                v_bias = bool(qk_bias)  # b_qkv nonzero => v bias nonzero path
                for t in range(TOKC):
                    for lo, hi in ((0, 512), (512, VW)):
                        vps = mpsum.tile([128, 512], F32, tag="mpsum", name="vps")
                        w = hi - lo
                        for cc in range(CC):
                            nc.tensor.matmul(
                                vps[:, :w],
                                xTb[:, cc, t * 128 : (t + 1) * 128],
                                wv_sb[:, cc, lo:hi],
                                start=(cc == 0),
                                stop=(cc == CC - 1 and not v_bias),
                            )
                        if v_bias:
                            # bias + per-head ones-columns via rank-1 update
                            nc.tensor.matmul(
                                vps[:, :w],
                                ones_sb[:, :],
                                bvaug_sb[:, lo:hi],
                                start=False,
                                stop=True,
                            )
                        nc.scalar.activation(v_sb[:, t, lo:hi], vps[:, :w], mybir.ActivationFunctionType.Copy)
                if not v_bias:
                    # fill each head's ones-column with a single strided DMA
                    nc.sync.dma_start(
                        v_sb.rearrange("p t (h a) -> p t h a", a=HDA)[:, :, :, HD],
                        vones_d[:],
                    )
                return xTb, v_sb

            def emit_heads(b, xTb, v_sb):
                # densely packed attn^T [C, N]: head h occupies rows
                # HD*h .. HD*h+HD; every row is written (no junk partitions).
                attnT = attn_pool.tile([128, CC, N], mmdt, tag="attnT", name="attnT")

                def qkproj_steps(h):
                    """Generator: head h's Q^T/K^T projection as 8 emission
                    steps of 3 matmuls each (plus the PSUM->SBUF copy when a
                    (f, tq) accumulation group completes). Interleaving these
                    into the previous head's attention loop keeps the PE fed
                    while ScalarE works through the exps."""
                    wqkh = wqkh_pool.tile([128, CC, 2 * HD], mmdt, tag="wqkh")
                    nc.scalar.dma_start(
                        wqkh[:], wqk_re[:, :, 2 * HD * h : 2 * HD * (h + 1)]
                    )
                    qkt = qkt_pool.tile([128, 2, N], mmdt, tag="qkt")
                    seq = [(f, tq, cc) for f in range(2) for tq in range(QH) for cc in range(CC)]
                    qps = None
                    for step in range(8):
                        for f, tq, cc in seq[3 * step : 3 * step + 3]:
                            if cc == 0:
                                qps = mpsum.tile(
                                    [128, 512], F32, tag="mpsum", name="qps"
                                )
                            nc.tensor.matmul(
                                qps[:HD, :],
                                wqkh[:, cc, HD * f : HD * (f + 1)],
                                xTb[:, cc, tq * 512 : (tq + 1) * 512],
                                start=(cc == 0),
                                stop=(cc == CC - 1),
                            )
                            if cc == CC - 1:
                                dst = qkt[:HD, f, tq * 512 : (tq + 1) * 512]
                                if qk_bias:
                                    nc.scalar.activation(
                                        dst,
                                        qps[:HD, :],
                                        mybir.ActivationFunctionType.Copy,
                                        bias=bqk_sb[:, 2 * h + f : 2 * h + f + 1],
                                    )
                                else:
                                    nc.vector.tensor_copy(dst, qps[:HD, :])
                        yield qkt

                # head 0's projection runs unoverlapped; head h+1's is spread
                # across head h's attention inner loop.
                qkt_cur = None
                for qkt_cur in qkproj_steps(0):
                    pass

                for h in range(NH):
                    nxt_gen = qkproj_steps(h + 1) if h + 1 < NH else None
                    qkt_next = None

                    # --- attention: S^T blocks, exp, P~ @ [V | 1] ---
                    ops = [
                        opsum_pool.tile([128, 512], F32, tag="opsum", name="ops")
                        for _ in range(QH)
                    ]
                    for kc in range(KC):
                        st = spsum.tile([128, QH, 512], F32, tag="spsum", name="st")
                        for qh in range(QH):
                            nc.tensor.matmul(
                                st[:, qh, :],
                                qkt_cur[:HD, 1, kc * 128 : (kc + 1) * 128],
                                qkt_cur[:HD, 0, qh * 512 : (qh + 1) * 512],
                                start=True,
                                stop=True,
                            )
                        pt = pt_pool.tile([128, QH, 512], mmdt, tag="pt", name="pt")
                        nc.scalar.activation(pt[:], st[:], EXP, scale=SCALE)
                        for qh in range(QH):
                            nc.tensor.matmul(
                                ops[qh][:HDA, :],
                                v_sb[:, kc, HDA * h : HDA * (h + 1)],
                                pt[:, qh, :],
                                start=(kc == 0),
                                stop=(kc == KC - 1),
                            )
                        if nxt_gen is not None:
                            qkt_next = next(nxt_gen)

                    # --- normalize O^T by 1/rowsum and repack into attn^T ---
                    rb = rb_pool.tile([128, N], F32, tag="rb")
                    rd = rdram_pool.tile([1, N], F32, tag="rd", name="rd")
                    for qh in range(QH):
                        qs = slice(qh * 512, (qh + 1) * 512)
                        # free the PSUM accumulator ASAP
                        ot = ot_pool.tile([128, 512], F32, tag="ot", name="ot")
                        # reciprocal first: it heads the (DRAM-bounce) broadcast
                        # chain; the O^T copy overlaps with the bounce DMAs
                        nc.vector.reciprocal(rb[HD : HD + 1, qs], ops[qh][HD : HD + 1, :])
                        nc.vector.tensor_copy(ot[:HD, :], ops[qh][:HD, :])
                        nc.gpsimd.dma_start(rd[:, qs], rb[HD : HD + 1, qs])
                        src = rd[:, qs]
                        bsrc = bass.AP(
                            tensor=src.tensor,
                            offset=src.offset,
                            ap=[[0, HD]] + [list(x) for x in src.ap[1:]],
                        )
                        nc.gpsimd.dma_start(rb[:HD, qs], bsrc)
                        oN = on_pool.tile([128, 512], mmdt, tag="oN", name="oN")
                        nc.vector.tensor_tensor(
                            oN[:HD, :],
                            ot[:HD, :],
                            rb[:HD, qs],
                            mybir.AluOpType.mult,
                        )
                        # repack rows HD*h..HD*h+HD of attn^T (may straddle one
                        # 128-partition chunk boundary -> up to two DMAs)
                        r0 = HD * h
                        cc0, p0 = divmod(r0, 128)
                        len1 = min(HD, 128 - p0)
                        nc.gpsimd.dma_start(
                            attnT[p0 : p0 + len1, cc0, qs], oN[:len1, :]
                        )
                        if len1 < HD:
                            nc.gpsimd.dma_start(
                                attnT[0 : HD - len1, cc0 + 1, qs],
                                oN[len1:HD, :],
                            )

                    if qkt_next is not None:
                        qkt_cur = qkt_next

                return attnT

            def emit_proj(b, attnT, qcs=None):
                """Output projection: dense 6-chunk contraction."""
                for qc in qcs if qcs is not None else range(QC):
                    out_sb = out_pool.tile([128, C], F32, tag="out", name="out_sb")
                    for n in range(2):
                        pps = mpsum.tile([128, 512], F32, tag="mpsum", name="pps")
                        ns = slice(n * 384, (n + 1) * 384)
                        for cc in range(CC):
                            nc.tensor.matmul(
                                pps[:, :384],
                                attnT[:, cc, qc * 128 : (qc + 1) * 128],
                                wp_sb[:, cc, ns],
                                start=(cc == 0),
                                stop=(cc == CC - 1 and not p_bias),
                            )
                        if p_bias:
                            nc.tensor.matmul(
                                pps[:, :384],
                                ones_sb[:, :],
                                bp_sb[:, ns],
                                start=False,
                                stop=True,
                            )
                        nc.scalar.activation(out_sb[:, ns], pps[:, :384], mybir.ActivationFunctionType.Copy)
                    nc.sync.dma_start(y_d[b, qc * 128 : (qc + 1) * 128, :], out_sb[:])

            # Emission (≈ static engine) order V0 H0 V1 P0 H1 P1: batch 1's V
            # projection fills the PE while batch 0's normalize tail drains,
            # and proj(0) fills it while batch 1's first heads project.
            xTb0, v0 = emit_vproj(0)
            at0 = emit_heads(0, xTb0, v0)
            xTb1, v1 = emit_vproj(1)
            # wp is first used by proj(0), ~20us after this point lands
            nc.scalar.dma_start(wp_sb[:], wp_re[:])
            at1 = emit_heads(1, xTb1, v1)
            # proj(0) is deferred past heads(1): it fills the PE while batch
            # 1's last normalize chain drains, and proj(1) backfills proj(0)'s
            # own epilogue.
            emit_proj(0, at0)
            emit_proj(1, at1)

    nc.compile()
    _BUILD_CACHE[key] = nc
    return nc


def _prep_shared(w_qkv, b_qkv, w_proj, b_proj):
    """Host-side weight rearrangement shared by all cores."""
    w_qkv = np.ascontiguousarray(w_qkv, dtype=np.float32)
    w_proj = np.ascontiguousarray(w_proj, dtype=np.float32)
    b_qkv = np.asarray(b_qkv, dtype=np.float32)
    b_proj = np.asarray(b_proj, dtype=np.float32)

    # wqk: [C, 2*NH*HD] with column 2*HD*h + HD*f + j = w_qkv row C*f + HD*h + j
    wqk = w_qkv[: 2 * C].reshape(2, NH, HD, C)  # [f, h, j, c]
    wqk_arr = np.ascontiguousarray(
        np.transpose(wqk, (3, 1, 0, 2)).reshape(C, 2 * NH * HD)
    )

    # wv: [C, NH*(HD+1)] with a zero ones-column slot per head
    wv = w_qkv[2 * C :].reshape(NH, HD, C)  # [h, j, c]
    wv_aug = np.zeros((C, NH, HDA), dtype=np.float32)
    wv_aug[:, :, :HD] = np.transpose(wv, (2, 0, 1))
    wv_aug = np.ascontiguousarray(wv_aug.reshape(C, VW))

    # wp: plain transpose [c_in, c_out]
    wp_t = np.ascontiguousarray(w_proj.T)

    # bvaug: v-bias interleaved with 1.0 at each head's ones-column
    bvaug = np.zeros((1, NH, HDA), dtype=np.float32)
    bvaug[0, :, :HD] = b_qkv[2 * C :].reshape(NH, HD)
    bvaug[0, :, HD] = 1.0
    bvaug = bvaug.reshape(1, VW)

    ones = np.ones((1, 128), dtype=np.float32)
    vones = np.ones((128, TOKC, NH), dtype=np.float32)

    qk_bias = bool(np.any(b_qkv[: 2 * C] != 0.0))
    p_bias = bool(np.any(b_proj != 0.0))
    extra = {}
    if qk_bias:
        # [HD, 2*NH] col 2h+f = bias of (f, h)
        bqk = b_qkv[: 2 * C].reshape(2, NH, HD)  # [f, h, j]
        extra["bqk"] = np.ascontiguousarray(
            np.transpose(bqk, (2, 1, 0)).reshape(HD, 2 * NH)
        )
    if p_bias:
        extra["bp"] = np.ascontiguousarray(b_proj.reshape(1, C))

    return wqk_arr, wv_aug, wp_t, bvaug, ones, vones, qk_bias, p_bias, extra


def kernel(x, w_qkv, b_qkv, w_proj, b_proj, H=32, W=32):
    x = np.asarray(x, dtype=np.float32)
    assert x.shape == (B, N, C), x.shape
    assert int(H) * int(W) == N

    wqk_arr, wv_aug, wp_t, bvaug, ones, vones, qk_bias, p_bias, extra = _prep_shared(
        w_qkv, b_qkv, w_proj, b_proj
    )
    nc = _build(qk_bias, p_bias)

    in_maps = []
    for c in range(NCORES):
        xc = x[BPC * c : BPC * (c + 1)].reshape(BPC * N, C)
        xT = np.ascontiguousarray(xc.T)  # [C, BPC*N]
        m = {
            "xT": xT,
            "wqk": wqk_arr,
            "wv": wv_aug,
            "wp": wp_t,
            "bvaug": bvaug,
            "ones": ones,
            "vones": vones,
        }
        m.update(extra)
        in_maps.append(m)

    trace = os.environ.get("KERNEL_TRACE") == "1"
    res = run_bass_kernel_spmd(
        nc, in_maps, core_ids=list(range(NCORES)), trace=trace
    )
    if trace:
        kernel.last_results = res
        print("exec_time_ns:", res.exec_time_ns, "mean:", res.mean_exec_time_ns)
        if res.instructions_and_trace:
            print("trace:", res.instructions_and_trace[1])

    out = np.empty((B, N, C), dtype=np.float32)
    for c in range(NCORES):
        out[BPC * c : BPC * (c + 1)] = res.results[c]["y"]
    return out


if __name__ == "__main__":
    rng = np.random.default_rng(0)
    x = rng.standard_normal((B, N, C), dtype=np.float32)
    w_qkv = rng.standard_normal((3 * C, C), dtype=np.float32) / np.sqrt(C)
    b_qkv = np.zeros(3 * C, np.float32)
    w_proj = rng.standard_normal((C, C), dtype=np.float32) / np.sqrt(C)
    b_proj = np.zeros(C, np.float32)
    y = kernel(x, w_qkv, b_qkv, w_proj, b_proj)
    print("out", y.shape, y.dtype, float(np.abs(y).mean()))



# revision 3
# speedup vs baseline: 1.0494x; 1.0494x over previous
"""Trainium2 Bass kernel for multi-head global attention (the
"DeformableAttention" module whose relative-position-bias path is inactive).

Reference computation (per batch b):
    qkv = x @ w_qkv.T + b_qkv            # [N, 3C]
    q, k, v = split/reshape to [nh, N, hd]
    attn = softmax((q @ k.T) * hd**-0.5)
    out  = (attn @ v) merged heads       # [N, C]
    y    = out @ w_proj.T + b_proj

Sharding: data-parallel over batch B=16 across 8 NeuronCores (2 batches/core).
No collectives.

Device-side design (per core, per batch):
  * All matmul operands are bf16 (weights cast on host, on-device tensors
    produced in bf16 by the PSUM-evacuation copies); accumulation stays fp32
    in PSUM. End-to-end quantization error ~6e-3 relative.
  * x is staged pre-transposed (xT, [C, tokens]) so every matmul contraction
    dim lands on SBUF partitions without any on-device transpose.
  * Q^T, K^T ([hd, N]) are produced per-head straight from the QKV projection
    (head-sized M=96 stationary tiles); V in natural [N, nh*(hd+1)] layout
    with an interleaved ones-column per head (memset).
  * Scores are computed transposed (S^T[k, q] blocks), softmax's exp runs on
    ScalarE with the 1/sqrt(hd) scale fused, and the row-sums fall out of the
    P~ @ [V | 1] matmul for free (row hd of the PSUM output).
  * O^T is copied out of PSUM immediately (frees the accumulator), normalized
    by the broadcast reciprocal row-sum (partition-broadcast via a DRAM
    bounce + stride-0 DMA), and DMA-repacked into a dense [C, N] attn^T
    buffer (partition-shifting SBUF->SBUF DMA).
  * Output projection contracts attn^T against w_proj.T in 6 dense 128-chunks,
    producing final [token, C] tiles in natural layout for direct DMA out.

Set KERNEL_MM_DT=f32r / f32 for higher-precision matmuls instead of bf16.
"""

import os
import sys

sys.path.insert(0, "/opt/trn_rl_repo")

# The Bass->PJRT execution path needs jax to discover the axon-tunneled
# NeuronCores; a stray JAX_PLATFORMS=cpu (e.g. set for a jax reference run)
# would hide them. Only effective if jax hasn't been imported yet.
if "jax" not in sys.modules and "axon" not in os.environ.get("JAX_PLATFORMS", "axon"):
    os.environ.pop("JAX_PLATFORMS", None)

import ml_dtypes
import numpy as np

import concourse.bass as bass
import concourse.mybir as mybir
import concourse.tile as tile
from concourse import bacc
from concourse.bass_utils import run_bass_kernel_spmd

# Problem constants (hardcoded per the task contract).
B, N, C = 16, 1024, 768
NH, HD = 8, 96
NCORES = 8
BPC = B // NCORES  # batches per core = 2
CC = C // 128  # contraction chunks of 128 = 6
KC = N // 128  # key chunks per batch = 8
QH = N // 512  # query halves = 2
TOKC = N // 128  # token chunks for V projection = 8
QC = N // 128  # query chunks for output projection = 8
HDA = HD + 1  # head dim + ones column = 97
VW = NH * HDA  # augmented V width = 776
SCALE = float(HD) ** -0.5

F32 = mybir.dt.float32
BF16_NP = ml_dtypes.bfloat16

_BUILD_CACHE = {}


def _mm_dt():
    return {
        "f32": (mybir.dt.float32, np.float32),
        "f32r": (mybir.dt.float32r, np.float32),
    }.get(os.environ.get("KERNEL_MM_DT", "bf16"), (mybir.dt.bfloat16, BF16_NP))


def _build(qk_bias: bool, p_bias: bool):
    """Build + compile the single-core Bass program (shared SPMD across cores)."""
    knobs = tuple(
        int(os.environ.get(k, d))
        for k, d in (
            ("PT_BUFS", 3),
            ("QKT_BUFS", 2),
            ("WQKH_BUFS", 2),
            ("SP_BUFS", 2),
            ("OP_BUFS", 2),
            ("MP_BUFS", 2),
            ("RB_BUFS", 1),
            ("OUT_BUFS", 2),
            ("OT_BUFS", 2),
            ("ON_BUFS", 2),
            ("X_BUFS", 2),
            ("V_BUFS", 2),
        )
    )
    key = (qk_bias, p_bias, os.environ.get("KERNEL_MM_DT", "bf16"), knobs)
    if key in _BUILD_CACHE:
        return _BUILD_CACHE[key]
    ptb, qktb, wqkhb, spb, opb, mpb, rbb, outb, otb, onb, xb, vb = knobs

    mmdt, _ = _mm_dt()

    nc = bacc.Bacc("TRN2", target_bir_lowering=False, debug=False)

    xT_d = nc.dram_tensor("xT", [C, BPC * N], mmdt, kind="ExternalInput")
    wqk_d = nc.dram_tensor("wqk", [C, 2 * NH * HD], mmdt, kind="ExternalInput")
    wv_d = nc.dram_tensor("wv", [C, VW], mmdt, kind="ExternalInput")
    wp_d = nc.dram_tensor("wp", [C, C], mmdt, kind="ExternalInput")
    if qk_bias:
        bvaug_d = nc.dram_tensor("bvaug", [1, VW], mmdt, kind="ExternalInput")
        ones_d = nc.dram_tensor("ones", [1, 128], mmdt, kind="ExternalInput")
        bqk_d = nc.dram_tensor("bqk", [HD, 2 * NH], F32, kind="ExternalInput")
    elif p_bias:
        ones_d = nc.dram_tensor("ones", [1, 128], mmdt, kind="ExternalInput")
    if p_bias:
        bp_d = nc.dram_tensor("bp", [1, C], mmdt, kind="ExternalInput")
    y_d = nc.dram_tensor("y", [BPC, N, C], F32, kind="ExternalOutput")

    xT_re = xT_d.rearrange("(o p) t -> p o t", p=128)
    wqk_re = wqk_d.rearrange("(o p) f -> p o f", p=128)
    wv_re = wv_d.rearrange("(o p) f -> p o f", p=128)
    wp_re = wp_d.rearrange("(o p) f -> p o f", p=128)

    EXP = mybir.ActivationFunctionType.Exp
    COPY = mybir.ActivationFunctionType.Copy

    with tile.TileContext(nc) as tc:
        with (
            tc.tile_pool(name="wpool", bufs=1) as wpool,
            tc.tile_pool(name="wqkh_pool", bufs=wqkhb) as wqkh_pool,
            tc.tile_pool(name="xpool", bufs=xb) as xpool,
            tc.tile_pool(name="qkt_pool", bufs=qktb) as qkt_pool,
            tc.tile_pool(name="vpool", bufs=vb) as vpool,
            tc.tile_pool(name="pt_pool", bufs=ptb) as pt_pool,
            tc.tile_pool(name="attn_pool", bufs=2) as attn_pool,
            tc.tile_pool(name="rb_pool", bufs=rbb) as rb_pool,
            tc.tile_pool(name="ot_pool", bufs=otb) as ot_pool,
            tc.tile_pool(name="on_pool", bufs=onb) as on_pool,
            tc.tile_pool(name="rdram_pool", bufs=2, space="DRAM") as rdram_pool,
            tc.tile_pool(name="out_pool", bufs=outb) as out_pool,
            tc.tile_pool(name="spsum", bufs=spb, space="PSUM") as spsum,
            tc.tile_pool(name="opsum_pool", bufs=opb, space="PSUM") as opsum_pool,
            tc.tile_pool(name="mpsum", bufs=mpb, space="PSUM") as mpsum,
        ):
            # --- resident weights/constants ---
            # wv is staged in small chunks so the first V-projection matmul
            # (which only needs chunk cc=0 of the low half) can start ~1us in.
            wv_sb = wpool.tile([128, CC, VW], mmdt, tag="wv")
            for lo, hi in ((0, 512), (512, VW)):
                for c0 in range(0, CC, 2):
                    nc.scalar.dma_start(
                        wv_sb[:, c0 : c0 + 2, lo:hi], wv_re[:, c0 : c0 + 2, lo:hi]
                    )
            # wp is not needed until the first output projection (~100us in);
            # issue its load after the first batch's V projection to keep the
            # startup-critical DMAs (x, wv, wqk head 0) ahead of it.
            wp_sb = wpool.tile([128, CC, C], mmdt, tag="wp")
            if qk_bias:
                bvaug_sb = wpool.tile([1, VW], mmdt, tag="bvaug")
                nc.scalar.dma_start(bvaug_sb[:], bvaug_d[:])
                bqk_sb = wpool.tile([HD, 2 * NH], F32, tag="bqk")
                nc.scalar.dma_start(bqk_sb[:], bqk_d[:])
            if qk_bias or p_bias:
                ones_sb = wpool.tile([1, 128], mmdt, tag="ones")
                nc.scalar.dma_start(ones_sb[:], ones_d[:])
            if p_bias:
                bp_sb = wpool.tile([1, C], mmdt, tag="bp")
                nc.scalar.dma_start(bp_sb[:], bp_d[:])

            def emit_vproj(b):
                """Stage batch b's x^T and project V (ones-augmented)."""
                xTb = xpool.tile([128, CC, N], mmdt, tag="xTb", name="xTb")
                # batch 0 feeds the first matmuls: stage a small leading chunk
                # so the V projection of token block 0 starts immediately.
                bounds = (0, 128, 512, N) if b == 0 else (0, 512, N)
                for lo, hi in zip(bounds, bounds[1:]):
                    nc.sync.dma_start(
                        xTb[:, :, lo:hi],
                        xT_re[:, :, b * N + lo : b * N + hi],
                    )
                v_sb = vpool.tile([128, TOKC, VW], mmdt, tag="v", name="v_sb")
                v_bias = bool(qk_bias)  # b_qkv nonzero => v bias nonzero path
                # low column halves first: the first 8 groups only need the
                # low half of wv, giving the high-half DMAs ~10us to land.
                for lo, hi in ((0, 512), (512, VW)):
                    for t in range(TOKC):
                        vps = mpsum.tile([128, 512], F32, tag="mpsum", name="vps")
                        w = hi - lo
                        for cc in range(CC):
                            nc.tensor.matmul(
                                vps[:, :w],
                                xTb[:, cc, t * 128 : (t + 1) * 128],
                                wv_sb[:, cc, lo:hi],
                                start=(cc == 0),
                                stop=(cc == CC - 1 and not v_bias),
                            )
                        if v_bias:
                            # bias + per-head ones-columns via rank-1 update
                            nc.tensor.matmul(
                                vps[:, :w],
                                ones_sb[:, :],
                                bvaug_sb[:, lo:hi],
                                start=False,
                                stop=True,
                            )
                        nc.scalar.activation(v_sb[:, t, lo:hi], vps[:, :w], COPY)
                if not v_bias:
                    # fill each head's ones-column (single strided memset)
                    nc.gpsimd.memset(
                        v_sb.rearrange("p t (h a) -> p t h a", a=HDA)[:, :, :, HD],
                        1.0,
                    )
                return xTb, v_sb

            def emit_heads(b, xTb, v_sb):
                # densely packed attn^T [C, N]: head h occupies rows
                # HD*h .. HD*h+HD; every row is written (no junk partitions).
                attnT = attn_pool.tile([128, CC, N], mmdt, tag="attnT", name="attnT")

                def qkproj_steps(h):
                    """Generator: head h's Q^T/K^T projection as 8 emission
                    steps of 3 matmuls each (plus the PSUM->SBUF copy when a
                    (f, tq) accumulation group completes). Interleaving these
                    into the previous head's attention loop keeps the PE fed
                    while ScalarE works through the exps."""
                    wqkh = wqkh_pool.tile([128, CC, 2 * HD], mmdt, tag="wqkh")
                    nc.scalar.dma_start(
                        wqkh[:], wqk_re[:, :, 2 * HD * h : 2 * HD * (h + 1)]
                    )
                    qkt = qkt_pool.tile([128, 2, N], mmdt, tag="qkt")
                    seq = [(f, tq, cc) for f in range(2) for tq in range(QH) for cc in range(CC)]
                    qps = None
                    for step in range(8):
                        for f, tq, cc in seq[3 * step : 3 * step + 3]:
                            if cc == 0:
                                qps = mpsum.tile(
                                    [128, 512], F32, tag="mpsum", name="qps"
                                )
                            nc.tensor.matmul(
                                qps[:HD, :],
                                wqkh[:, cc, HD * f : HD * (f + 1)],
                                xTb[:, cc, tq * 512 : (tq + 1) * 512],
                                start=(cc == 0),
                                stop=(cc == CC - 1),
                            )
                            if cc == CC - 1:
                                dst = qkt[:HD, f, tq * 512 : (tq + 1) * 512]
                                if qk_bias:
                                    nc.scalar.activation(
                                        dst,
                                        qps[:HD, :],
                                        COPY,
                                        bias=bqk_sb[:, 2 * h + f : 2 * h + f + 1],
                                    )
                                else:
                                    nc.vector.tensor_copy(dst, qps[:HD, :])
                        yield qkt

                def emit_norm(h, qh, ops_qh, rb, rd):
                    """Normalize O^T[qh] by 1/rowsum and repack into attn^T.
                    Emitted immediately after the last PV matmul of (h, qh) so
                    the PSUM accumulator recycles before the next head needs
                    it."""
                    qs = slice(qh * 512, (qh + 1) * 512)
                    ot = ot_pool.tile([128, 512], mmdt, tag="ot", name="ot")
                    # reciprocal first: it heads the (DRAM-bounce) broadcast
                    # chain; the O^T copy overlaps with the bounce DMAs
                    with nc.allow_low_precision(
                        reason="1/rowsum in bf16: ~2e-3 rel, inside tolerance"
                    ):
                        nc.vector.reciprocal(
                            rb[HD : HD + 1, qs], ops_qh[HD : HD + 1, :]
                        )
                    if qh == 0:
                        # ScalarE has a hole here (next exp waits on scores)
                        nc.scalar.activation(ot[:HD, :], ops_qh[:HD, :], COPY)
                    else:
                        nc.vector.tensor_copy(ot[:HD, :], ops_qh[:HD, :])
                    nc.gpsimd.dma_start(rd[:, qs], rb[HD : HD + 1, qs])
                    src = rd[:, qs]
                    bsrc = bass.AP(
                        tensor=src.tensor,
                        offset=src.offset,
                        ap=[[0, HD]] + [list(x) for x in src.ap[1:]],
                    )
                    nc.gpsimd.dma_start(rb[:HD, qs], bsrc)
                    oN = on_pool.tile([128, 512], mmdt, tag="oN", name="oN")
                    nc.vector.tensor_tensor(
                        oN[:HD, :],
                        ot[:HD, :],
                        rb[:HD, qs],
                        mybir.AluOpType.mult,
                    )
                    # repack rows HD*h..HD*h+HD of attn^T (may straddle one
                    # 128-partition chunk boundary -> up to two DMAs)
                    r0 = HD * h
                    cc0, p0 = divmod(r0, 128)
                    len1 = min(HD, 128 - p0)
                    nc.gpsimd.dma_start(attnT[p0 : p0 + len1, cc0, qs], oN[:len1, :])
                    if len1 < HD:
                        nc.gpsimd.dma_start(
                            attnT[0 : HD - len1, cc0 + 1, qs],
                            oN[len1:HD, :],
                        )

                # head 0's projection runs unoverlapped; head h+1's is spread
                # across head h's attention inner loop.
                qkt_cur = None
                for qkt_cur in qkproj_steps(0):
                    pass

                for h in range(NH):
                    nxt_gen = qkproj_steps(h + 1) if h + 1 < NH else None
                    qkt_next = None

                    # --- attention: S^T blocks, exp, P~ @ [V | 1] ---
                    ops = [
                        opsum_pool.tile([128, 512], F32, tag="opsum", name="ops")
                        for _ in range(QH)
                    ]
                    rb = rb_pool.tile([128, N], mmdt, tag="rb")
                    rd = rdram_pool.tile([1, N], mmdt, tag="rd", name="rd")
                    for kc in range(KC):
                        st = spsum.tile([128, QH, 512], F32, tag="spsum", name="st")
                        for qh in range(QH):
                            nc.tensor.matmul(
                                st[:, qh, :],
                                qkt_cur[:HD, 1, kc * 128 : (kc + 1) * 128],
                                qkt_cur[:HD, 0, qh * 512 : (qh + 1) * 512],
                                start=True,
                                stop=True,
                            )
                        pt = pt_pool.tile([128, QH, 512], mmdt, tag="pt", name="pt")
                        nc.scalar.activation(pt[:], st[:], EXP, scale=SCALE)
                        for qh in range(QH):
                            nc.tensor.matmul(
                                ops[qh][:HDA, :],
                                v_sb[:, kc, HDA * h : HDA * (h + 1)],
                                pt[:, qh, :],
                                start=(kc == 0),
                                stop=(kc == KC - 1),
                            )
                            if kc == KC - 1:
                                # emit the normalize chain for qh right after
                                # its final PV: the PSUM buf frees while the
                                # other qh / next head's scores run on the PE
                                emit_norm(h, qh, ops[qh], rb, rd)
                        if nxt_gen is not None:
                            qkt_next = next(nxt_gen)

                    if qkt_next is not None:
                        qkt_cur = qkt_next

                return attnT

            def emit_proj(b, attnT, qcs=None):
                """Output projection: dense 6-chunk contraction."""
                for qc in qcs if qcs is not None else range(QC):
                    out_sb = out_pool.tile([128, C], F32, tag="out", name="out_sb")
                    for n in range(2):
                        pps = mpsum.tile([128, 512], F32, tag="mpsum", name="pps")
                        ns = slice(n * 384, (n + 1) * 384)
                        for cc in range(CC):
                            nc.tensor.matmul(
                                pps[:, :384],
                                attnT[:, cc, qc * 128 : (qc + 1) * 128],
                                wp_sb[:, cc, ns],
                                start=(cc == 0),
                                stop=(cc == CC - 1 and not p_bias),
                            )
                        if p_bias:
                            nc.tensor.matmul(
                                pps[:, :384],
                                ones_sb[:, :],
                                bp_sb[:, ns],
                                start=False,
                                stop=True,
                            )
                        nc.scalar.activation(out_sb[:, ns], pps[:, :384], COPY)
                        # per-half output DMA: shortens the end-of-kernel drain
                        nc.sync.dma_start(
                            y_d[b, qc * 128 : (qc + 1) * 128, ns], out_sb[:, ns]
                        )

            # Emission (~ static engine) order V0 H0 V1 P0 H1 P1: batch 1's V
            # projection fills the PE while batch 0's normalize tail drains,
            # and proj(0) fills it while batch 1's first heads project.
            xTb0, v0 = emit_vproj(0)
            at0 = emit_heads(0, xTb0, v0)
            xTb1, v1 = emit_vproj(1)
            # wp is first used by proj(0), ~20us after this point lands
            nc.scalar.dma_start(wp_sb[:], wp_re[:])
            at1 = emit_heads(1, xTb1, v1)
            # proj(0) is deferred past heads(1): it fills the PE while batch
            # 1's last normalize chain drains, and proj(1) backfills proj(0)'s
            # own epilogue.
            emit_proj(0, at0)
            emit_proj(1, at1)

    nc.compile()
    _BUILD_CACHE[key] = nc
    return nc


def _prep_shared(w_qkv, b_qkv, w_proj, b_proj):
    """Host-side weight rearrangement shared by all cores."""
    _, np_dt = _mm_dt()
    w_qkv = np.ascontiguousarray(w_qkv, dtype=np.float32)
    w_proj = np.ascontiguousarray(w_proj, dtype=np.float32)
    b_qkv = np.asarray(b_qkv, dtype=np.float32)
    b_proj = np.asarray(b_proj, dtype=np.float32)

    # wqk: [C, 2*NH*HD] with column 2*HD*h + HD*f + j = w_qkv row C*f + HD*h + j
    wqk = w_qkv[: 2 * C].reshape(2, NH, HD, C)  # [f, h, j, c]
    wqk_arr = np.ascontiguousarray(
        np.transpose(wqk, (3, 1, 0, 2)).reshape(C, 2 * NH * HD).astype(np_dt)
    )

    # wv: [C, NH*(HD+1)] with a zero ones-column slot per head
    wv = w_qkv[2 * C :].reshape(NH, HD, C)  # [h, j, c]
    wv_aug = np.zeros((C, NH, HDA), dtype=np.float32)
    wv_aug[:, :, :HD] = np.transpose(wv, (2, 0, 1))
    wv_aug = np.ascontiguousarray(wv_aug.reshape(C, VW).astype(np_dt))

    # wp: plain transpose [c_in, c_out]
    wp_t = np.ascontiguousarray(w_proj.T.astype(np_dt))

    qk_bias = bool(np.any(b_qkv[: 2 * C] != 0.0))
    p_bias = bool(np.any(b_proj != 0.0))
    extra = {}
    if qk_bias:
        # bvaug: v-bias interleaved with 1.0 at each head's ones-column
        bvaug = np.zeros((1, NH, HDA), dtype=np.float32)
        bvaug[0, :, :HD] = b_qkv[2 * C :].reshape(NH, HD)
        bvaug[0, :, HD] = 1.0
        extra["bvaug"] = bvaug.reshape(1, VW).astype(np_dt)
        # [HD, 2*NH] col 2h+f = bias of (f, h)
        bqk = b_qkv[: 2 * C].reshape(2, NH, HD)  # [f, h, j]
        extra["bqk"] = np.ascontiguousarray(
            np.transpose(bqk, (2, 1, 0)).reshape(HD, 2 * NH)
        )
    if qk_bias or p_bias:
        extra["ones"] = np.ones((1, 128), dtype=np_dt)
    if p_bias:
        extra["bp"] = np.ascontiguousarray(b_proj.reshape(1, C).astype(np_dt))

    return wqk_arr, wv_aug, wp_t, qk_bias, p_bias, extra


def kernel(x, w_qkv, b_qkv, w_proj, b_proj, H=32, W=32):
    x = np.asarray(x, dtype=np.float32)
    assert x.shape == (B, N, C), x.shape
    assert int(H) * int(W) == N

    _, np_dt = _mm_dt()
    wqk_arr, wv_aug, wp_t, qk_bias, p_bias, extra = _prep_shared(
        w_qkv, b_qkv, w_proj, b_proj
    )
    nc = _build(qk_bias, p_bias)

    in_maps = []
    for c in range(NCORES):
        xc = x[BPC * c : BPC * (c + 1)].reshape(BPC * N, C)
        xT = np.ascontiguousarray(xc.T.astype(np_dt))  # [C, BPC*N]
        m = {
            "xT": xT,
            "wqk": wqk_arr,
            "wv": wv_aug,
            "wp": wp_t,
        }
        m.update(extra)
        in_maps.append(m)

    trace = os.environ.get("KERNEL_TRACE") == "1"
    res = run_bass_kernel_spmd(
        nc, in_maps, core_ids=list(range(NCORES)), trace=trace
    )
    if trace:
        kernel.last_results = res
        print("exec_time_ns:", res.exec_time_ns, "mean:", res.mean_exec_time_ns)
        if res.instructions_and_trace:
            print("trace:", res.instructions_and_trace[1])

    out = np.empty((B, N, C), dtype=np.float32)
    for c in range(NCORES):
        out[BPC * c : BPC * (c + 1)] = res.results[c]["y"]
    return out


if __name__ == "__main__":
    rng = np.random.default_rng(0)
    x = rng.standard_normal((B, N, C), dtype=np.float32)
    w_qkv = rng.standard_normal((3 * C, C), dtype=np.float32) / np.sqrt(C)
    b_qkv = np.zeros(3 * C, np.float32)
    w_proj = rng.standard_normal((C, C), dtype=np.float32) / np.sqrt(C)
    b_proj = np.zeros(C, np.float32)
    y = kernel(x, w_qkv, b_qkv, w_proj, b_proj)
    print("out", y.shape, y.dtype, float(np.abs(y).mean()))


# revision 47
# speedup vs baseline: 1.1417x; 1.0880x over previous
"""Trainium2 Bass kernel for multi-head global attention (the
"DeformableAttention" module whose relative-position-bias path is inactive).

Reference computation (per batch b):
    qkv = x @ w_qkv.T + b_qkv            # [N, 3C]
    q, k, v = split/reshape to [nh, N, hd]
    attn = softmax((q @ k.T) * hd**-0.5)
    out  = (attn @ v) merged heads       # [N, C]
    y    = out @ w_proj.T + b_proj

Sharding: data-parallel over batch B=16 across 8 NeuronCores (2 batches/core).
No collectives.

Device-side design (per core, per batch):
  * All matmul operands are bf16 (weights cast on host, on-device tensors
    produced in bf16 by the PSUM-evacuation copies); accumulation stays fp32
    in PSUM. End-to-end quantization error ~6e-3 relative (tolerance 2e-2).
  * x is staged pre-transposed (xT, [C, tokens]) so every matmul contraction
    dim lands on SBUF partitions without any on-device transpose.
  * Q^T/K^T are projected head-PAIR-wise with dense 128-column stationary
    tiles (3 loads per 2 heads instead of 4): the packed PSUM tiles are
    evacuated with partition-aligned copies (to the per-head planes where
    already aligned, else to a staging tile) plus 4 partition-shifting
    SBUF->SBUF DMAs per (pair, token-half). Odd heads live at partition base
    32, even heads at base 0. This removes the 25% PE waste of 96-row
    stationaries.
  * V is projected in natural [N, nh*(hd+1)] layout with an interleaved
    ones-column per head (memset).
  * Scores are computed transposed (S^T[k, q] blocks) ONE key-chunk ahead of
    the P~ @ [V | 1] matmuls, hiding the exp (ScalarE) latency; the row-sums
    fall out of the augmented-V matmul for free.
  * O^T is evacuated immediately (frees PSUM), normalized by the broadcast
    reciprocal row-sum (partition-broadcast via a DRAM bounce + stride-0
    DMA), and DMA-repacked into a dense [C, N] attn^T buffer.
  * Output projection contracts attn^T against w_proj.T in 6 dense 128-chunks.
  * Emission interleave keeps the PE dense: batch 0's head-pair projections
    ride inside the previous pair's attention steps; batch 1's V projection
    rides inside batch 0's last two heads; proj(0) rides inside batch 1's
    last two heads; proj(1) drains at the end against both PSUM pools.

Set KERNEL_MM_DT=f32r / f32 for higher-precision matmuls instead of bf16.
"""

import os
import sys

sys.path.insert(0, "/opt/trn_rl_repo")

# The Bass->PJRT execution path needs jax to discover the axon-tunneled
# NeuronCores; a stray JAX_PLATFORMS=cpu (e.g. set for a jax reference run)
# would hide them. Only effective if jax hasn't been imported yet.
if "jax" not in sys.modules and "axon" not in os.environ.get("JAX_PLATFORMS", "axon"):
    os.environ.pop("JAX_PLATFORMS", None)

import ml_dtypes
import numpy as np

import concourse.bass as bass
import concourse.mybir as mybir
import concourse.tile as tile
from concourse import bacc
from concourse.bass_utils import run_bass_kernel_spmd

# Problem constants (hardcoded per the task contract).
B, N, C = 16, 1024, 768
NH, HD = 8, 96
NP = NH // 2  # head pairs = 4
NCORES = 8
BPC = B // NCORES  # batches per core = 2
CC = C // 128  # contraction chunks of 128 = 6
KC = N // 128  # key chunks per batch = 8
QH = N // 512  # query halves = 2
TOKC = N // 128  # token chunks for V projection = 8
QC = N // 128  # query chunks for output projection = 8
HDA = HD + 1  # head dim + ones column = 97
VW = NH * HDA  # augmented V width = 776
SCALE = float(HD) ** -0.5

F32 = mybir.dt.float32
BF16_NP = ml_dtypes.bfloat16

_BUILD_CACHE = {}


def _mm_dt():
    return {
        "f32": (mybir.dt.float32, np.float32),
        "f32r": (mybir.dt.float32r, np.float32),
    }.get(os.environ.get("KERNEL_MM_DT", "bf16"), (mybir.dt.bfloat16, BF16_NP))


def _build(qk_bias: bool, p_bias: bool):
    """Build + compile the single-core Bass program (shared SPMD across cores)."""
    knobs = tuple(
        int(os.environ.get(k, d))
        for k, d in (
            ("PT_BUFS", 3),
            ("QKT_BUFS", 4),
            ("WQKH_BUFS", 2),
            ("SP_BUFS", 2),
            ("OP_BUFS", 2),
            ("MP_BUFS", 2),
            ("RB_BUFS", 2),
            ("OUT_BUFS", 3),
            ("OT_BUFS", 2),
            ("ON_BUFS", 2),
            ("X_BUFS", 2),
            ("V_BUFS", 2),
            ("STAGE_BUFS", 2),
        )
    )
    key = (qk_bias, p_bias, os.environ.get("KERNEL_MM_DT", "bf16"), knobs)
    if key in _BUILD_CACHE:
        return _BUILD_CACHE[key]
    (ptb, qktb, wqkhb, spb, opb, mpb, rbb, outb, otb, onb, xb, vb, stb) = knobs

    mmdt, _ = _mm_dt()

    nc = bacc.Bacc("TRN2", target_bir_lowering=False, debug=False)

    xT_d = nc.dram_tensor("xT", [C, BPC * N], mmdt, kind="ExternalInput")
    wqk_d = nc.dram_tensor("wqk", [C, 2 * NH * HD], mmdt, kind="ExternalInput")
    wv_d = nc.dram_tensor("wv", [C, VW], mmdt, kind="ExternalInput")
    wp_d = nc.dram_tensor("wp", [C, C], mmdt, kind="ExternalInput")
    if qk_bias:
        bvaug_d = nc.dram_tensor("bvaug", [1, VW], mmdt, kind="ExternalInput")
        # per-(pair, j-tile) partition-packed q/k biases, rank-1-added in PSUM
        bqkp_d = nc.dram_tensor("bqkp", [1, NP * 3 * 128], mmdt, kind="ExternalInput")
    if qk_bias or p_bias:
        ones_d = nc.dram_tensor("ones", [1, 512], mmdt, kind="ExternalInput")
    if p_bias:
        bp_d = nc.dram_tensor("bp", [1, C], mmdt, kind="ExternalInput")
    y_d = nc.dram_tensor("y", [BPC, N, C], F32, kind="ExternalOutput")

    xT_re = xT_d.rearrange("(o p) t -> p o t", p=128)
    wqk_re = wqk_d.rearrange("(o p) f -> p o f", p=128)
    wv_re = wv_d.rearrange("(o p) f -> p o f", p=128)
    wp_re = wp_d.rearrange("(o p) f -> p o f", p=128)

    EXP = mybir.ActivationFunctionType.Exp
    COPY = mybir.ActivationFunctionType.Copy

    with tile.TileContext(nc) as tc:
        with (
            tc.tile_pool(name="wpool", bufs=1) as wpool,
            tc.tile_pool(name="wqkh_pool", bufs=wqkhb) as wqkh_pool,
            tc.tile_pool(name="xpool", bufs=xb) as xpool,
            tc.tile_pool(name="qkt_pool", bufs=qktb) as qkt_pool,
            tc.tile_pool(name="stage_pool", bufs=stb) as stage_pool,
            tc.tile_pool(name="vpool", bufs=vb) as vpool,
            tc.tile_pool(name="pt_pool", bufs=ptb) as pt_pool,
            tc.tile_pool(name="attn_pool", bufs=2) as attn_pool,
            tc.tile_pool(name="rb_pool", bufs=rbb) as rb_pool,
            tc.tile_pool(name="ot_pool", bufs=otb) as ot_pool,
            tc.tile_pool(name="on_pool", bufs=onb) as on_pool,
            tc.tile_pool(name="rdram_pool", bufs=2, space="DRAM") as rdram_pool,
            tc.tile_pool(name="out_pool", bufs=outb) as out_pool,
            tc.tile_pool(name="spsum", bufs=spb, space="PSUM") as spsum,
            tc.tile_pool(name="opsum_pool", bufs=opb, space="PSUM") as opsum_pool,
            tc.tile_pool(name="mpsum", bufs=mpb, space="PSUM") as mpsum,
        ):
            # --- resident weights/constants ---
            # wv is staged in small chunks so the first V-projection matmul
            # (which only needs chunk cc=0 of the low half) can start early.
            # Spread across the three DGE generators so startup descriptor
            # generation doesn't serialize on one device. x's first chunk
            # heads the SP queue; wv's first chunk follows it there (the
            # scalar queue pays a ~1.3us activation-table load first, so its
            # DMAs start late).
            xTb0 = xpool.tile([128, CC, N], mmdt, tag="xTb", name="xTb")
            nc.sync.dma_start(xTb0[:, :, 0:256], xT_re[:, :, 0:256])
            wv_sb = wpool.tile([128, CC, VW], mmdt, tag="wv")
            nc.sync.dma_start(wv_sb[:, 0:2, 0:512], wv_re[:, 0:2, 0:512])
            nc.sync.dma_start(xTb0[:, :, 256:N], xT_re[:, :, 256:N])
            nc.gpsimd.dma_start(wv_sb[:, 2:4, 0:512], wv_re[:, 2:4, 0:512])
            nc.scalar.dma_start(wv_sb[:, 4:CC, 0:512], wv_re[:, 4:CC, 0:512])
            nc.gpsimd.dma_start(wv_sb[:, 0:3, 512:VW], wv_re[:, 0:3, 512:VW])
            nc.scalar.dma_start(wv_sb[:, 3:CC, 512:VW], wv_re[:, 3:CC, 512:VW])
            # wp is not needed until the first output projection (~100us in);
            # issue its load after the first batch's V projection to keep the
            # startup-critical DMAs (x, wv, wqk pair 0) ahead of it.
            wp_sb = wpool.tile([128, CC, C], mmdt, tag="wp")
            if qk_bias:
                bvaug_sb = wpool.tile([1, VW], mmdt, tag="bvaug")
                nc.scalar.dma_start(bvaug_sb[:], bvaug_d[:])
                bqkp_sb = wpool.tile([1, NP * 3 * 128], mmdt, tag="bqkp")
                nc.scalar.dma_start(bqkp_sb[:], bqkp_d[:])
            if qk_bias or p_bias:
                ones_sb = wpool.tile([1, 512], mmdt, tag="ones")
                nc.scalar.dma_start(ones_sb[:], ones_d[:])
            if p_bias:
                bp_sb = wpool.tile([1, C], mmdt, tag="bp")
                nc.scalar.dma_start(bp_sb[:], bp_d[:])

            def stage_x(b):
                """Allocate + DMA batch b's x^T (hoistable for prefetch)."""
                xTb = xpool.tile([128, CC, N], mmdt, tag="xTb", name="xTb")
                bounds = (0, 256, N) if b == 0 else (0, 512, N)
                for lo, hi in zip(bounds, bounds[1:]):
                    nc.sync.dma_start(
                        xTb[:, :, lo:hi], xT_re[:, :, b * N + lo : b * N + hi]
                    )
                return xTb

            def make_vproj(b, xTb=None):
                """Generator yields the 16 V-projection groups (low column
                halves first, so the first 8 groups only need the low half of
                wv)."""
                if xTb is None:
                    xTb = stage_x(b)
                v_sb = vpool.tile([128, TOKC, VW], mmdt, tag="v", name="v_sb")
                v_bias = bool(qk_bias)  # b_qkv nonzero => v bias nonzero path

                def vmm(vps, t, cc, lo, hi):
                    nc.tensor.matmul(
                        vps[:, : hi - lo],
                        xTb[:, cc, t * 128 : (t + 1) * 128],
                        wv_sb[:, cc, lo:hi],
                        start=(cc == 0),
                        stop=(cc == CC - 1 and not v_bias),
                    )

                def vtail(vps, t, lo, hi):
                    if v_bias:
                        nc.tensor.matmul(
                            vps[:, : hi - lo],
                            ones_sb[:, :128],
                            bvaug_sb[:, lo:hi],
                            start=False,
                            stop=True,
                        )
                    nc.scalar.activation(v_sb[:, t, lo:hi], vps[:, : hi - lo], COPY)

                def gen():
                    if b == 0:
                        # tokens 0/1 interleaved per contraction chunk: the PE
                        # has two matmuls ready per arriving wv/x DMA chunk,
                        # halving the startup arrival stalls
                        vp0 = mpsum.tile([128, 512], F32, tag="mpsum", name="vps")
                        vp1 = mpsum.tile([128, 512], F32, tag="mpsum", name="vps")
                        for cc in range(CC):
                            vmm(vp0, 0, cc, 0, 512)
                            vmm(vp1, 1, cc, 0, 512)
                        vtail(vp0, 0, 0, 512)
                        yield
                        vtail(vp1, 1, 0, 512)
                        yield
                    for lo, hi in ((0, 512), (512, VW)):
                        for t in range(2 if (b == 0 and lo == 0) else 0, TOKC):
                            vps = mpsum.tile([128, 512], F32, tag="mpsum", name="vps")
                            for cc in range(CC):
                                vmm(vps, t, cc, lo, hi)
                            vtail(vps, t, lo, hi)
                            yield
                    if not v_bias:
                        # fill each head's ones-column (single strided memset)
                        nc.gpsimd.memset(
                            v_sb.rearrange("p t (h a) -> p t h a", a=HDA)[:, :, :, 0],
                            1.0,
                        )

                return xTb, v_sb, gen()

            def make_pairproj(b, p, xTb, evac_act=False):
                """Project Q^T/K^T for heads (2p, 2p+1) with three dense
                128-column stationary tiles per contraction chunk. Returns the
                two per-head plane tiles + a generator of ~14 emission quanta
                (3 matmuls each).

                Packed PSUM partition layout per column tile j:
                  j0 = [q0 rows 0:96 | k0 rows 0:32]
                  j1 = [k0 rows 32:96 | q1 rows 0:64]
                  j2 = [q1 rows 64:96 | k1 rows 0:96]
                All head planes live at partition base 0 (engine APs must fit
                the naturally-aligned block of their base partition, so e.g. a
                96-partition span at base 32 is not encodable). q0's span is
                evacuated straight to its plane; everything else goes via a
                staging tile (aligned copies) + partition-shifting SBUF->SBUF
                DMAs, which have no partition constraints."""
                wqkp = wqkh_pool.tile([128, CC, 384], mmdt, tag="wqkh")
                nc.scalar.dma_start(wqkp[:], wqk_re[:, :, 384 * p : 384 * (p + 1)])
                qkt0 = qkt_pool.tile([128, 2, N], mmdt, tag="qkt", name="qkt0")
                qkt1 = qkt_pool.tile([128, 2, N], mmdt, tag="qkt", name="qkt1")
                stage = stage_pool.tile([128, 3, N], mmdt, tag="stage", name="stage")

                def gen():
                    for tq in range(2):
                        ts = slice(tq * 512, (tq + 1) * 512)
                        for j in range(3):
                            qps = mpsum.tile([128, 512], F32, tag="mpsum", name="qps")
                            for cc in range(CC):
                                nc.tensor.matmul(
                                    qps[:, :],
                                    wqkp[:, cc, 128 * j : 128 * (j + 1)],
                                    xTb[:, cc, ts],
                                    start=(cc == 0),
                                    stop=(cc == CC - 1 and not qk_bias),
                                )
                                if cc == 2:
                                    yield
                            if qk_bias:
                                o = (p * 3 + j) * 128
                                nc.tensor.matmul(
                                    qps[:, :],
                                    bqkp_sb[:, o : o + 128],
                                    ones_sb[:, :],
                                    start=False,
                                    stop=True,
                                )
                            # evac_act: split the two-copy evacuations across
                            # ScalarE+DVE so the PSUM buf frees in one copy
                            # latency (used in dense blocks where ScalarE has
                            # slack; inside attention windows the exps own it)
                            def cp2(dst, src):
                                if evac_act:
                                    nc.scalar.activation(dst, src, COPY)
                                else:
                                    nc.vector.tensor_copy(dst, src)

                            if j == 0:
                                cp2(qkt0[0:HD, 0, ts], qps[0:HD, :])
                                nc.vector.tensor_copy(
                                    stage[96:128, 0, ts], qps[96:128, :]
                                )
                            elif j == 1:
                                nc.vector.tensor_copy(stage[:, 1, ts], qps[:, :])
                            else:
                                cp2(stage[:, 2, ts], qps[:, :])
                            yield
                        # partition-shifting repack of the misaligned spans
                        # (HWDGE via the SP queue, which is idle during the
                        # attention windows)
                        nc.sync.dma_start(qkt0[0:32, 1, ts], stage[96:128, 0, ts])
                        nc.sync.dma_start(qkt0[32:HD, 1, ts], stage[0:64, 1, ts])
                        nc.sync.dma_start(qkt1[0:64, 0, ts], stage[64:128, 1, ts])
                        nc.sync.dma_start(qkt1[64:HD, 0, ts], stage[0:32, 2, ts])
                        nc.sync.dma_start(qkt1[0:HD, 1, ts], stage[32:128, 2, ts])
                        yield

                return (qkt0, qkt1), gen()

            def pull(fill, k=1):
                if fill is not None:
                    for _ in range(k):
                        if next(fill, None) is None:
                            break

            # ones column for the rank-1 PSUM broadcast in the tail normalize
            onesc = wpool.tile([1, 128], mmdt, tag="onesc")
            nc.gpsimd.memset(onesc[:], 1.0)

            def emit_norm(h, qh, ops_qh, rb, rd, attnT, fast=False):
                """Normalize O^T[qh] by 1/rowsum and repack into attn^T.
                Emitted immediately after the last PV matmul of (h, qh) so the
                PSUM accumulator recycles before the next head needs it."""
                # The ones column sits at index 0 of each head's V block, so
                # the row-sums land on PSUM partition 0 and O^T occupies rows
                # 1..HDA. Engine ops span the full [0:HDA) block (legal at
                # base 0); row 0 of the product is sum*rinv junk that the
                # repack DMA simply skips.
                qs = slice(qh * 512, (qh + 1) * 512)
                ot = ot_pool.tile([128, 512], mmdt, tag="ot", name="ot")
                # reciprocal first: it heads the broadcast chain; the O^T copy
                # overlaps with it
                with nc.allow_low_precision(
                    reason="1/rowsum in bf16: ~2e-3 rel, inside tolerance"
                ):
                    nc.vector.reciprocal(rb[0:1, qs], ops_qh[0:1, :])
                if qh == 0:
                    # ScalarE has a hole here (next exp waits on scores)
                    nc.scalar.activation(ot[:HDA, :], ops_qh[:HDA, :], COPY)
                else:
                    nc.vector.tensor_copy(ot[:HDA, :], ops_qh[:HDA, :])
                oN = on_pool.tile([128, 512], mmdt, tag="oN", name="oN")
                if fast:
                    # tail variant: partition-broadcast 1/rowsum via a rank-1
                    # matmul into PSUM (~0.2us on the otherwise-idle PE)
                    # instead of the ~4.5us DRAM-bounce DMA round trip, so the
                    # final head's attn^T rows land before the last output
                    # projection needs them
                    rbb = mpsum.tile([128, 512], F32, tag="mpsum", name="rbb")
                    nc.tensor.matmul(
                        rbb[:HDA, :], onesc[:1, :HDA], rb[0:1, qs],
                        start=True, stop=True,
                    )
                    nc.vector.tensor_tensor(
                        oN[:HDA, :], ot[:HDA, :], rbb[:HDA, :],
                        mybir.AluOpType.mult,
                    )
                else:
                    nc.gpsimd.dma_start(rd[:, qs], rb[0:1, qs])
                    src = rd[:, qs]
                    bsrc = bass.AP(
                        tensor=src.tensor,
                        offset=src.offset,
                        ap=[[0, HDA]] + [list(x) for x in src.ap[1:]],
                    )
                    nc.gpsimd.dma_start(rb[:HDA, qs], bsrc)
                    nc.vector.tensor_tensor(
                        oN[:HDA, :], ot[:HDA, :], rb[:HDA, qs],
                        mybir.AluOpType.mult,
                    )
                # repack rows HD*h..HD*h+HD of attn^T (may straddle one
                # 128-partition chunk boundary -> up to two DMAs). HWDGE via
                # the SP queue: the gpsimd queue is saturated by the
                # bounce/broadcast chains and would delay the repacks that
                # gate the output projections.
                r0 = HD * h
                cc0, p0 = divmod(r0, 128)
                len1 = min(HD, 128 - p0)
                nc.sync.dma_start(attnT[p0 : p0 + len1, cc0, qs], oN[1 : 1 + len1, :])
                if len1 < HD:
                    nc.sync.dma_start(
                        attnT[0 : HD - len1, cc0 + 1, qs], oN[1 + len1 : 1 + HD, :]
                    )

            def emit_head(h, qkt, v_sb, attnT, fill, npull=1, fast_norm=False):
                """One head's attention: scores run one key-chunk ahead of the
                PV accumulation so the exp latency on ScalarE is hidden behind
                the interleave quanta + previous chunk's PV."""
                ops = [
                    opsum_pool.tile([128, 512], F32, tag="opsum", name="ops")
                    for _ in range(QH)
                ]
                rb = rb_pool.tile([128, N], mmdt, tag="rb")
                rd = rdram_pool.tile([1, N], mmdt, tag="rd", name="rd")
                pts = [None] * KC
                hs = slice(0, HD)

                def emit_pv(kc):
                    for qh in range(QH):
                        nc.tensor.matmul(
                            ops[qh][:HDA, :],
                            v_sb[:, kc, HDA * h : HDA * (h + 1)],
                            pts[kc][:, qh, :],
                            start=(kc == 0),
                            stop=(kc == KC - 1),
                        )
                        if kc == KC - 1:
                            emit_norm(h, qh, ops[qh], rb, rd, attnT, fast=fast_norm)

                for kc in range(KC):
                    st = spsum.tile([128, QH, 512], F32, tag="spsum", name="st")
                    for qh in range(QH):
                        nc.tensor.matmul(
                            st[:, qh, :],
                            qkt[hs, 1, kc * 128 : (kc + 1) * 128],
                            qkt[hs, 0, qh * 512 : (qh + 1) * 512],
                            start=True,
                            stop=True,
                        )
                    pts[kc] = pt_pool.tile([128, QH, 512], mmdt, tag="pt", name="pt")
                    nc.scalar.activation(pts[kc][:], st[:], EXP, scale=SCALE)
                    pull(fill, npull)
                    if kc >= 1:
                        emit_pv(kc - 1)
                emit_pv(KC - 1)

            def run_heads(b, v_sb, attnT, pair0, window_makers):
                cur = pair0
                for w in range(4):
                    nxt, fill, k = window_makers[w]()
                    for hh in range(2):
                        h = 2 * w + hh
                        emit_head(
                            h, cur[hh], v_sb, attnT, fill, k,
                            fast_norm=(b == 1 and h == NH - 1),
                        )
                    if fill is not None:
                        for _ in fill:  # drain leftover quanta
                            pass
                    if nxt is not None:
                        cur = nxt
                return attnT

            def make_outproj(b, attnT, pools, ydma=None):
                """Output projection: dense 6-chunk contraction; generator
                yields the 16 (token-chunk, half) groups."""

                def gen():
                    it = 0
                    for qc in range(QC):
                        out_sb = out_pool.tile([128, C], F32, tag="out", name="out_sb")
                        for n in range(2):
                            pool = pools[it % len(pools)]
                            it += 1
                            pps = pool.tile(
                                [128, 512],
                                F32,
                                tag="opsum" if pool is opsum_pool else "mpsum",
                                name="pps",
                            )
                            ns = slice(n * 384, (n + 1) * 384)
                            for cc in range(CC):
                                nc.tensor.matmul(
                                    pps[:, :384],
                                    attnT[:, cc, qc * 128 : (qc + 1) * 128],
                                    wp_sb[:, cc, ns],
                                    start=(cc == 0),
                                    stop=(cc == CC - 1 and not p_bias),
                                )
                            if p_bias:
                                nc.tensor.matmul(
                                    pps[:, :384],
                                    ones_sb[:, :128],
                                    bp_sb[:, ns],
                                    start=False,
                                    stop=True,
                                )
                            nc.scalar.activation(out_sb[:, ns], pps[:, :384], COPY)
                            # per-half output DMA. proj(0) runs inside batch
                            # 1's last attention window: its y goes on the
                            # gpsimd queue so the heads' attn^T repacks (SP)
                            # don't queue behind it. proj(1) runs after all
                            # heads: SP is free and its HWDGE gen is faster.
                            (ydma or nc.gpsimd).dma_start(
                                y_d[b, qc * 128 : (qc + 1) * 128, ns], out_sb[:, ns]
                            )
                            yield

                return gen()

            # ---------------- emission schedule ----------------
            # prologue: batch 0's V projection with pair 0's Q/K projection
            # interleaved into its tail (2 quanta per group so each packed
            # column tile's PSUM accumulation completes within one V group —
            # half-finished accumulations would pin an mpsum buf and stall
            # the V pipeline)
            _, v0, vgen0 = make_vproj(0, xTb=xTb0)
            pair0_b0, pgen = make_pairproj(0, 0, xTb0, evac_act=True)
            for i, _ in enumerate(vgen0):
                if i >= 4:
                    pull(pgen, 2)
            for _ in pgen:
                pass

            slot = {}
            from itertools import chain as _chain

            def mk_pair(bb, pp, xTb, pre=None):
                def make():
                    if pre is not None:
                        pre()
                    tiles, g = make_pairproj(bb, pp, xTb)
                    return tiles, g, 1

                return make

            def mk_w3b0():
                # x1 was staged a window earlier (prefetch). Window 3 hosts
                # batch 1's pair-0 Q/K projection THEN its V projection,
                # front-loaded at 3 quanta/step: pair 0's shift DMAs land long
                # before batch 1's first scores, and V1 + its ones-memset
                # complete a few steps before batch 1's first PV.
                _, v1, vgen1 = make_vproj(1, xTb=slot["x1"])
                slot["v1"] = v1
                tiles, g01 = make_pairproj(1, 0, slot["x1"], evac_act=True)
                slot["pair0b1"] = tiles
                return None, _chain(g01, vgen1), 3

            attnT0 = attn_pool.tile([128, CC, N], mmdt, tag="attnT", name="attnT")
            run_heads(
                0,
                v0,
                attnT0,
                pair0_b0,
                [
                    mk_pair(0, 1, xTb0),
                    mk_pair(0, 2, xTb0),
                    # stage batch 1's x alongside pair 3's projection so the
                    # DMA lands well before V1 consumes it in the next window
                    mk_pair(0, 3, xTb0, pre=lambda: slot.__setitem__("x1", stage_x(1))),
                    mk_w3b0,
                ],
            )

            xTb1, v1 = slot["x1"], slot["v1"]
            nc.scalar.dma_start(wp_sb[:], wp_re[:])  # used ~40us later by proj(0)

            attnT1 = attn_pool.tile([128, CC, N], mmdt, tag="attnT", name="attnT")
            run_heads(
                1,
                v1,
                attnT1,
                slot["pair0b1"],
                [
                    mk_pair(1, 1, xTb1),
                    mk_pair(1, 2, xTb1),
                    mk_pair(1, 3, xTb1),
                    lambda: (None, make_outproj(0, attnT0, [mpsum]), 1),
                ],
            )

            # proj(1) drains at the end with both PSUM pools in rotation
            for _ in make_outproj(1, attnT1, [mpsum, opsum_pool], ydma=nc.sync):
                pass

    nc.compile()
    _BUILD_CACHE[key] = nc
    return nc


def _prep_shared(w_qkv, b_qkv, w_proj, b_proj):
    """Host-side weight rearrangement shared by all cores."""
    _, np_dt = _mm_dt()
    w_qkv = np.ascontiguousarray(w_qkv, dtype=np.float32)
    w_proj = np.ascontiguousarray(w_proj, dtype=np.float32)
    b_qkv = np.asarray(b_qkv, dtype=np.float32)
    b_proj = np.asarray(b_proj, dtype=np.float32)

    # wqk: [C, 2*NH*HD] with column 2*HD*h + HD*f + j = w_qkv row C*f + HD*h + j
    # (pair p's heads occupy the contiguous 384-column block [384p, 384p+384))
    wqk = w_qkv[: 2 * C].reshape(2, NH, HD, C)  # [f, h, j, c]
    wqk_arr = np.ascontiguousarray(
        np.transpose(wqk, (3, 1, 0, 2)).reshape(C, 2 * NH * HD).astype(np_dt)
    )

    # wv: [C, NH*(1+HD)] with a zero ones-column slot at index 0 per head
    # (leading slot => row-sums land on PSUM partition 0, see emit_norm)
    wv = w_qkv[2 * C :].reshape(NH, HD, C)  # [h, j, c]
    wv_aug = np.zeros((C, NH, HDA), dtype=np.float32)
    wv_aug[:, :, 1:] = np.transpose(wv, (2, 0, 1))
    wv_aug = np.ascontiguousarray(wv_aug.reshape(C, VW).astype(np_dt))

    # wp: plain transpose [c_in, c_out]
    wp_t = np.ascontiguousarray(w_proj.T.astype(np_dt))

    qk_bias = bool(np.any(b_qkv[: 2 * C] != 0.0))
    p_bias = bool(np.any(b_proj != 0.0))
    extra = {}
    if qk_bias:
        # bvaug: v-bias interleaved with 1.0 at each head's ones-column
        bvaug = np.zeros((1, NH, HDA), dtype=np.float32)
        bvaug[0, :, 1:] = b_qkv[2 * C :].reshape(NH, HD)
        bvaug[0, :, 0] = 1.0
        extra["bvaug"] = bvaug.reshape(1, VW).astype(np_dt)
        # bqkp: per-(pair, j-tile) partition-packed q/k biases, matching the
        # packed projection PSUM layout (see make_pairproj)
        bqk = b_qkv[: 2 * C].reshape(2, NH, HD)  # [f, h, j]
        bqkp = np.zeros((NP, 3, 128), dtype=np.float32)
        for p in range(NP):
            h0, h1 = 2 * p, 2 * p + 1
            bqkp[p, 0, 0:HD] = bqk[0, h0]
            bqkp[p, 0, HD:128] = bqk[1, h0, 0:32]
            bqkp[p, 1, 0:64] = bqk[1, h0, 32:HD]
            bqkp[p, 1, 64:128] = bqk[0, h1, 0:64]
            bqkp[p, 2, 0:32] = bqk[0, h1, 64:HD]
            bqkp[p, 2, 32:128] = bqk[1, h1]
        extra["bqkp"] = bqkp.reshape(1, NP * 3 * 128).astype(np_dt)
    if qk_bias or p_bias:
        extra["ones"] = np.ones((1, 512), dtype=np_dt)
    if p_bias:
        extra["bp"] = np.ascontiguousarray(b_proj.reshape(1, C).astype(np_dt))

    return wqk_arr, wv_aug, wp_t, qk_bias, p_bias, extra


def kernel(x, w_qkv, b_qkv, w_proj, b_proj, H=32, W=32):
    x = np.asarray(x, dtype=np.float32)
    assert x.shape == (B, N, C), x.shape
    assert int(H) * int(W) == N

    _, np_dt = _mm_dt()
    wqk_arr, wv_aug, wp_t, qk_bias, p_bias, extra = _prep_shared(
        w_qkv, b_qkv, w_proj, b_proj
    )
    nc = _build(qk_bias, p_bias)

    in_maps = []
    for c in range(NCORES):
        xc = x[BPC * c : BPC * (c + 1)].reshape(BPC * N, C)
        xT = np.ascontiguousarray(xc.T.astype(np_dt))  # [C, BPC*N]
        m = {
            "xT": xT,
            "wqk": wqk_arr,
            "wv": wv_aug,
            "wp": wp_t,
        }
        m.update(extra)
        in_maps.append(m)

    trace = os.environ.get("KERNEL_TRACE") == "1"
    res = run_bass_kernel_spmd(
        nc, in_maps, core_ids=list(range(NCORES)), trace=trace
    )
    if trace:
        kernel.last_results = res
        print("exec_time_ns:", res.exec_time_ns, "mean:", res.mean_exec_time_ns)
        if res.instructions_and_trace:
            print("trace:", res.instructions_and_trace[1])

    out = np.empty((B, N, C), dtype=np.float32)
    for c in range(NCORES):
        out[BPC * c : BPC * (c + 1)] = res.results[c]["y"]
    return out


if __name__ == "__main__":
    rng = np.random.default_rng(0)
    x = rng.standard_normal((B, N, C), dtype=np.float32)
    w_qkv = rng.standard_normal((3 * C, C), dtype=np.float32) / np.sqrt(C)
    b_qkv = np.zeros(3 * C, np.float32)
    w_proj = rng.standard_normal((C, C), dtype=np.float32) / np.sqrt(C)
    b_proj = np.zeros(C, np.float32)
    y = kernel(x, w_qkv, b_qkv, w_proj, b_proj)
    print("out", y.shape, y.dtype, float(np.abs(y).mean()))
